# revision 41
# baseline (speedup 1.0000x reference)
"""Trainium2 Bass kernel for nn_BaoCypherNet (tree-conv GNN).

Data-parallel over 8 NeuronCores: each core processes 256 trees.

Per tree, per layer l (C_l -> O_l, kernel 3, stride 3 over gathered triples):
  z[o, m] = sum_k sum_c W_k[o, c] * X[c, idx[3m+k]] + b[o]   (m = 1..127)
  x_cat = [0 | z]  -> layernorm over (O, 128) -> leaky_relu (layers 1, 2)

Node-major ("transposed") pipeline -- activations live as [node, channel]
tiles so the one-hot gather matmul needs NO transposes anywhere:
  - One-hot P[n, j] = (n == idx[j]) is host-built (bf16, pad columns
    zeroed) and DMA'd; j is k-major (j = k*128 + m).
  - gather: g[c, j] = sum_n x[n, c] * P[n, j] -- stationary = the
    node-major activation tile slice, moving = P.  PSUM -> SBUF copy.
  - convs (layers 1/2) run "reversed": stationary = gathered k-block
    g_k[c, m], moving = weights W_k[c, o]; output z^T[m, o] lands
    node-major, exactly the layout the next gather wants.  Layer 3 uses
    channel-major convs (stationary weights, N=512 over G=4 trees) since
    its output feeds the free-dim max-pool, not another gather.
  - bias = K=1 matmul (ones column [1,127] stationary) over node rows
    only; LN mean-subtract = one K=128 matmul (ones stationary, scaled
    per-row sums broadcast as moving operand).  P's pad columns are
    zero, so PSUM row 127 is exactly -mean after the shift: the node-0
    row of the next activation is leaky(-mean), written by one
    broadcast DVE copy.
  - LN *scale* is deferred for layers 1/2 (leaky(s*x) = s*leaky(x), next
    LN is scale-invariant up to eps); only layer 3 materializes stats,
    with b3 folded in via activation-bias on the Square and pooled path.
  - Pooling: max over PSUM cols, max(x+b3,0) handles the node-0 column;
    tiny MLP (W4 -> leaky -> W5) finishes on PE+DVE.
"""

import ml_dtypes
import numpy as np

import bass_rust as _bass_rust
import concourse.bass as bass
import concourse.mybir as mybir
from concourse.bass_utils import run_bass_kernel_spmd
from concourse.tile import TileContext

F32 = mybir.dt.float32
BF16 = mybir.dt.bfloat16

N_CORES = 8
B = 2048
BC = B // N_CORES  # trees per core
N = 128            # nodes (incl. zero-pad node 0)
M = N - 1          # conv output positions
G = 4              # trees per group
NGROUPS = BC // G
K1 = 256 * 128     # LN element counts per tree
K2 = 128 * 128
K3 = 64 * 128

_ALPHA = 0.01


def _ap(t_ap, extra_dims, offset_delta=0):
    """Build an AP on the same tensor with given free dims appended to the
    partition dim of `t_ap` (a full-tile AP)."""
    return bass.AP(
        tensor=t_ap.tensor,
        offset=t_ap.offset + offset_delta,
        ap=[t_ap.ap[0]] + list(extra_dims),
    )


def build_nc():
    nc = bass.Bass()

    g1_in = nc.dram_tensor("g1pre", [128, BC, 384], BF16, kind="ExternalInput")
    p_in = nc.dram_tensor("ponehot", [128, BC, 384], BF16, kind="ExternalInput")
    w1_in = nc.dram_tensor("w1r", [128, 3, 256], BF16, kind="ExternalInput")
    w2_in = nc.dram_tensor("w2t", [128, 3, 2, 128], BF16, kind="ExternalInput")
    w3_in = nc.dram_tensor("w3t", [128, 3, 64], BF16, kind="ExternalInput")
    br_in = nc.dram_tensor("biasrows", [1, 384], BF16, kind="ExternalInput")
    oc_in = nc.dram_tensor("onescol", [1, 128], BF16, kind="ExternalInput")
    bc_in = nc.dram_tensor("bcols", [128, 8], F32, kind="ExternalInput")
    ones128b_in = nc.dram_tensor("ones128b", [128, 128], BF16, kind="ExternalInput")
    ones128f_in = nc.dram_tensor("ones128f", [128, 128], F32, kind="ExternalInput")
    mlp_in = nc.dram_tensor("mlp_rhs", [66, 32], F32, kind="ExternalInput")
    w5_in = nc.dram_tensor("w5rep", [128, 32], F32, kind="ExternalInput")
    b5_in = nc.dram_tensor("b5rep", [128, 1], F32, kind="ExternalInput")
    out_dram = nc.dram_tensor("out", [BC, 1], F32, kind="ExternalOutput")

    with TileContext(nc) as tc:
        with (
            tc.tile_pool(name="const", bufs=1) as cp,
            tc.tile_pool(name="sb", bufs=4) as sb,
            tc.tile_pool(name="psz1", bufs=1, space="PSUM") as psz1,
            tc.tile_pool(name="pszB", bufs=1, space="PSUM") as pszB,
            tc.tile_pool(name="psg", bufs=4, space="PSUM") as psg,
            tc.tile_pool(name="pss", bufs=1, space="PSUM") as pss,
        ):
            # ---- constants ----
            w1r = cp.tile([128, 3, 256], BF16, tag="w1r")
            nc.sync.dma_start(out=w1r[:], in_=w1_in[:])
            w2t = cp.tile([128, 3, 2, 128], BF16, tag="w2t")
            nc.sync.dma_start(out=w2t[:], in_=w2_in[:])
            w3t = cp.tile([128, 3, 64], BF16, tag="w3t")
            nc.sync.dma_start(out=w3t[:], in_=w3_in[:])
            biasrows = cp.tile([1, 384], BF16, tag="biasrows")
            nc.sync.dma_start(out=biasrows[:], in_=br_in[:])
            onescol = cp.tile([1, 128], BF16, tag="onescol")
            nc.sync.dma_start(out=onescol[:], in_=oc_in[:])
            bcols = cp.tile([128, 8], F32, tag="bcols")
            nc.sync.dma_start(out=bcols[:], in_=bc_in[:])
            ones128b = cp.tile([128, 128], BF16, tag="ones128b")
            nc.sync.dma_start(out=ones128b[:], in_=ones128b_in[:])
            ones128f = cp.tile([128, 128], F32, tag="ones128f")
            nc.sync.dma_start(out=ones128f[:], in_=ones128f_in[:])
            mlp_rhs = cp.tile([66, 32], F32, tag="mlp_rhs")
            nc.sync.dma_start(out=mlp_rhs[:], in_=mlp_in[:])
            w5rep = cp.tile([128, 32], F32, tag="w5rep")
            nc.sync.dma_start(out=w5rep[:], in_=w5_in[:])
            b5rep = cp.tile([128, 1], F32, tag="b5rep")
            nc.sync.dma_start(out=b5rep[:], in_=b5_in[:])

            for g in range(NGROUPS):
                t0 = g * G
                # ---- inputs for this group ----
                # g1 is host-gathered (im2col): g1[c, t, j] = x1[c, idx[j]]
                g1 = sb.tile([128, G, 384], BF16, tag="g1")
                nc.sync.dma_start(out=g1[:], in_=g1_in[:, t0:t0 + G, :])
                P = sb.tile([128, G, 384], BF16, tag="P")
                nc.sync.dma_start(out=P[:], in_=p_in[:, t0:t0 + G, :])

                # ---- layer 1 convs (reversed): z1T[m, o] per tree ----
                z1T = psz1.tile([128, G, 256], F32, tag="z1")
                for t in range(G):
                    for k in range(3):
                        nc.tensor.matmul(
                            z1T[:, t, :],
                            g1[:, t, k * 128:(k + 1) * 128],
                            w1r[:, k, :],
                            start=(t % 2 == 0 and k == 0), stop=(k == 2),
                            skip_group_check=True)

                # LN1 mean: per-row sums (per tree, overlapping the other
                # trees' convs) -> fused -mean via ones MM
                s1 = sb.tile([128, G], F32, tag="s1")
                for t in range(G):
                    nc.vector.tensor_reduce(
                        s1[:, t:t + 1], _ap(z1T[:], [[1, 256]], t * 256),
                        axis=mybir.AxisListType.X, op=mybir.AluOpType.add)
                s1n = sb.tile([128, G], BF16, tag="s1n")
                with nc.allow_low_precision(reason="mean shift in bf16"):
                    nc.vector.tensor_scalar(
                        s1n[:], s1[:], -1.0 / K1, bcols[:, 5:6],
                        mybir.AluOpType.mult, mybir.AluOpType.add)
                # bias (rows 0..126 only) + mean shift (all 128 rows)
                z1Tr = z1T[0:127, :, :]
                for h in range(2):
                    nc.tensor.matmul(
                        bass.AP(tensor=z1Tr.tensor,
                                offset=z1Tr.offset + h * 512,
                                ap=[z1Tr.ap[0], [256, 2], [1, 256]]),
                        onescol[:, 0:127],
                        _ap(biasrows[:], [[0, 2], [1, 256]]),
                        start=False, stop=False, skip_group_check=True)
                    nc.tensor.matmul(
                        _ap(z1T[:], [[256, 2], [1, 256]], h * 512),
                        ones128b[:],
                        bass.AP(tensor=s1n.tensor,
                                offset=s1n[:].offset + h * 2,
                                ap=[s1n[:].ap[0], [1, 2], [0, 256]]),
                        start=False, stop=True, skip_group_check=True)

                # X2 node-major [row, t, c]: row r = node r+1 for r<127;
                # row 127 = node 0 = leaky(-mean) (PSUM pad row, no bias).
                x2 = sb.tile([128, G, 256], BF16, tag="x2")
                nc.scalar.activation(
                    x2[:], z1T[:],
                    mybir.ActivationFunctionType.Prelu,
                    bias=0.0, scale=1.0, alpha=_ALPHA)

                # ---- layer 2 gathers (stationary = x2 chunks) ----
                g2 = sb.tile([128, 2, 3, G, 128], BF16, tag="g2")
                for t in range(G):
                    for j in range(2):
                        gp = psg.tile([128, 384], F32, tag="g",
                                      name=f"g2p{t}{j}")
                        nc.tensor.matmul(
                            gp[:], x2[:, t, j * 128:(j + 1) * 128],
                            P[:, t, :], start=True, stop=True)
                        g2out = _ap(g2[:], [[G * 128, 3], [1, 128]],
                                    j * 3 * G * 128 + t * 128)
                        if t == 3:
                            with nc.allow_low_precision(reason="copy bf16"):
                                nc.vector.tensor_copy(g2out, gp[:])
                        else:
                            nc.scalar.activation(
                                g2out, gp[:],
                                mybir.ActivationFunctionType.Copy,
                                bias=0.0, scale=1.0)

                # ---- layer 2 convs (reversed): z2T[m, o] ----
                z2T = pszB.tile([128, G, 128], F32, tag="zB", name="z2T")
                for t in range(G):
                    for j in range(2):
                        for k in range(3):
                            nc.tensor.matmul(
                                z2T[:, t, :], g2[:, j, k, t, :],
                                w2t[:, k, j, :],
                                start=(t == 0 and j == 0 and k == 0),
                                stop=(k == 2 and j == 1),
                                skip_group_check=True)

                # LN2 mean (per tree)
                s2 = sb.tile([128, G], F32, tag="s2")
                for t in range(G):
                    nc.vector.tensor_reduce(
                        s2[:, t:t + 1], _ap(z2T[:], [[1, 128]], t * 128),
                        axis=mybir.AxisListType.X, op=mybir.AluOpType.add)
                s2n = sb.tile([128, G], BF16, tag="s2n")
                with nc.allow_low_precision(reason="mean shift in bf16"):
                    nc.vector.tensor_scalar(
                        s2n[:], s2[:], -1.0 / K2, bcols[:, 6:7],
                        mybir.AluOpType.mult, mybir.AluOpType.add)
                z2Tr = z2T[0:127, :, :]
                nc.tensor.matmul(
                    bass.AP(tensor=z2Tr.tensor, offset=z2Tr.offset,
                            ap=[z2Tr.ap[0], [128, G], [1, 128]]),
                    onescol[:, 0:127],
                    _ap(biasrows[:], [[0, G], [1, 128]], 256),
                    start=False, stop=False, skip_group_check=True)
                nc.tensor.matmul(
                    _ap(z2T[:], [[128, G], [1, 128]]),
                    ones128b[:],
                    _ap(s2n[:], [[1, G], [0, 128]]),
                    start=False, stop=True, skip_group_check=True)

                # X3 node-major, same row permutation as X2
                x3 = sb.tile([128, G, 128], BF16, tag="x3")
                nc.scalar.activation(
                    x3[:], z2T[:],
                    mybir.ActivationFunctionType.Prelu,
                    bias=0.0, scale=1.0, alpha=_ALPHA)

                # ---- layer 3: gather, channel-major conv, LN, max-pool ----
                g3 = sb.tile([128, 3, G, 128], BF16, tag="g3")
                for t in range(G):
                    gp = psg.tile([128, 384], F32, tag="g", name=f"g3p{t}")
                    nc.tensor.matmul(gp[:], x3[:, t, :], P[:, t, :],
                                     start=True, stop=True)
                    g3out = _ap(g3[:], [[G * 128, 3], [1, 128]], t * 128)
                    if t >= 2:
                        with nc.allow_low_precision(reason="copy bf16"):
                            nc.vector.tensor_copy(g3out, gp[:])
                    else:
                        nc.scalar.activation(
                            g3out, gp[:], mybir.ActivationFunctionType.Copy,
                            bias=0.0, scale=1.0)

                z3 = pszB.tile([64, G, 128], F32, tag="zB", name="z3")
                z3_all = _ap(z3[:], [[128, G], [1, 128]])
                for k in range(3):
                    rhs = _ap(g3[:], [[128, G], [1, 128]], k * G * 128)
                    nc.tensor.matmul(z3_all, w3t[:, k, :], rhs,
                                     start=(k == 0), stop=(k == 2))

                # LN3 stats on z3+b3 (b3 via ACT bias / post-corrections)
                z3v = _ap(z3[:], [[128, G], [1, 127]])
                s3 = sb.tile([64, 2, G], F32, tag="s3")
                nc.vector.tensor_reduce(
                    s3[:, 0, :], z3v, axis=mybir.AxisListType.X,
                    op=mybir.AluOpType.add)
                nc.vector.tensor_scalar(
                    s3[:, 0, :], s3[:, 0, :], bcols[0:64, 4:5], None,
                    mybir.AluOpType.add)
                sq = sb.tile([64, G, 127], F32, tag="sq")
                nc.scalar.activation(
                    sq[:], z3v, mybir.ActivationFunctionType.Square,
                    bias=bcols[0:64, 3:4], scale=1.0)
                nc.vector.tensor_reduce(
                    s3[:, 1, :], sq[:], axis=mybir.AxisListType.X,
                    op=mybir.AluOpType.add)
                ps3 = pss.tile([128, 2, G], F32, tag="pss", name="ps3")
                nc.tensor.matmul(
                    ps3[:], ones128f[0:64, :],
                    bass.AP(tensor=s3.tensor, offset=s3[:].offset,
                            ap=[s3[:].ap[0], [1, 2 * G]]),
                    start=True, stop=True)
                # mean3 = S/K3; nm3 = -mean3; var = SS/(K3-1) - K3/(K3-1)*mean^2
                mean3 = sb.tile([128, G], F32, tag="mean3")
                nc.vector.tensor_scalar(
                    mean3[:], ps3[:, 0, :], 1.0 / K3, None, mybir.AluOpType.mult)
                nm3 = sb.tile([128, G], F32, tag="nm3")
                nc.vector.tensor_scalar(
                    nm3[:], mean3[:], -1.0, None, mybir.AluOpType.mult)
                m3sq = sb.tile([128, G], F32, tag="m3sq")
                nc.vector.tensor_tensor(
                    m3sq[:], mean3[:], mean3[:], mybir.AluOpType.mult)
                var3 = sb.tile([128, G], F32, tag="var3")
                nc.vector.tensor_scalar(
                    var3[:], m3sq[:], -float(K3) / (K3 - 1), None,
                    mybir.AluOpType.mult)
                ssn = sb.tile([128, G], F32, tag="ssn")
                nc.vector.tensor_scalar(
                    ssn[:], ps3[:, 1, :], 1.0 / (K3 - 1), None,
                    mybir.AluOpType.mult)
                nc.vector.tensor_tensor(
                    var3[:], var3[:], ssn[:], mybir.AluOpType.add)
                std3 = sb.tile([128, G], F32, tag="std3")
                nc.scalar.activation(
                    std3[:], var3[:], mybir.ActivationFunctionType.Sqrt,
                    bias=0.0, scale=1.0)
                nc.vector.tensor_scalar(
                    std3[:], std3[:], 1e-5, None, mybir.AluOpType.add)
                sinv3 = sb.tile([128, G], F32, tag="sinv3")
                nc.vector.reciprocal(sinv3[:], std3[:])

                # pooled = sinv3 * (max(max_m z3 + b3, 0) - mean3)
                pr = sb.tile([64, G], F32, tag="pr")
                nc.vector.tensor_reduce(
                    pr[:], z3v, axis=mybir.AxisListType.X,
                    op=mybir.AluOpType.max)
                paug = sb.tile([66, G], F32, tag="paug")
                nc.vector.memset(paug[64:66, :], 1.0)
                r1 = sb.tile([64, G], F32, tag="r1")
                nc.vector.tensor_scalar(
                    r1[:], pr[:], bcols[0:64, 3:4], 0.0,
                    mybir.AluOpType.add, mybir.AluOpType.max)
                r2 = sb.tile([64, G], F32, tag="r2")
                nc.vector.tensor_tensor(
                    r2[:], r1[:], nm3[0:64, :], mybir.AluOpType.add)
                nc.vector.tensor_tensor(
                    paug[0:64, :], r2[:], sinv3[0:64, :], mybir.AluOpType.mult)

                # h = leaky(W4 @ pooled + b4); out = h @ W5.T + b5
                ph = pss.tile([G, 32], F32, tag="pss", name="ph")
                nc.tensor.matmul(ph[:], paug[:, :], mlp_rhs[0:66, :],
                                 start=True, stop=True)
                h = sb.tile([G, 32], F32, tag="h")
                nc.scalar.activation(
                    h[:], ph[:], mybir.ActivationFunctionType.Prelu,
                    bias=0.0, scale=1.0, alpha=_ALPHA)
                prod = sb.tile([G, 32], F32, tag="prod")
                nc.vector.tensor_tensor(
                    prod[:], h[:], w5rep[0:G, :], mybir.AluOpType.mult)
                ov = sb.tile([G, 1], F32, tag="ov")
                nc.vector.tensor_reduce(
                    ov[:], prod[:], axis=mybir.AxisListType.X,
                    op=mybir.AluOpType.add)
                nc.vector.tensor_scalar(
                    ov[:], ov[:], b5rep[0:G, :], None, mybir.AluOpType.add)
                nc.sync.dma_start(out=out_dram[t0:t0 + G, :], in_=ov[:])

    _bass_rust.generate_event_semaphores(nc)
    nc.finalize()
    return nc


_NC_CACHE = None


def _get_nc():
    global _NC_CACHE
    if _NC_CACHE is None:
        _NC_CACHE = build_nc()
    return _NC_CACHE


def _prep_idx_flat(indexes: np.ndarray) -> np.ndarray:
    """indexes [B, 381] -> k-major [B, 384] int32 with pads = -1."""
    b = indexes.shape[0]
    idxk = np.full((b, 3, 128), -1, np.int32)
    tri = indexes.reshape(b, 127, 3).astype(np.int32)
    idxk[:, :, :127] = tri.transpose(0, 2, 1)
    return idxk.reshape(b, 384)


def kernel(trees, W1, b1, W2, b2, W3, b3, W4, b4, W5, b5, indexes):
    trees = np.asarray(trees, dtype=np.float32)
    indexes = np.asarray(indexes).astype(np.int64)
    W1 = np.asarray(W1, dtype=np.float32)
    W2 = np.asarray(W2, dtype=np.float32)
    W3 = np.asarray(W3, dtype=np.float32)
    W4 = np.asarray(W4, dtype=np.float32)
    W5 = np.asarray(W5, dtype=np.float32)
    b1 = np.asarray(b1, dtype=np.float32)
    b2 = np.asarray(b2, dtype=np.float32)
    b3 = np.asarray(b3, dtype=np.float32)
    b4 = np.asarray(b4, dtype=np.float32)
    b5 = np.asarray(b5, dtype=np.float32)

    nc = _get_nc()

    bf = ml_dtypes.bfloat16
    # replicated weight prep
    # w1r[c, k, o] = W1[o, c, k]
    w1r = np.ascontiguousarray(W1.transpose(1, 2, 0)).astype(bf)
    # w2t[p, k, j, o] = W2[o, j*128+p, k]
    w2t = np.ascontiguousarray(
        W2.reshape(128, 2, 128, 3).transpose(2, 3, 1, 0)).astype(bf)
    # w3t[c, k, o] = W3[o, c, k]
    w3t = np.ascontiguousarray(W3.transpose(1, 2, 0)).astype(bf)
    biasrows = np.zeros((1, 384), np.float32)
    biasrows[0, :256] = b1
    biasrows[0, 256:] = b2
    biasrows = biasrows.astype(bf)
    onescol = np.ones((1, 128), bf)
    # bias columns + scalar mean-bias corrections
    bcols = np.zeros((128, 8), np.float32)
    bcols[:64, 3] = b3
    bcols[:64, 4] = 127.0 * b3
    bcols[:, 5] = -127.0 * float(b1.sum()) / (K1 * 128)  # L1 shift mean-bias
    bcols[:, 6] = -127.0 * float(b2.sum()) / (K2 * 128)  # L2 shift mean-bias
    ones128b = np.ones((128, 128), bf)
    ones128f = np.ones((128, 128), np.float32)
    mlp_rhs = np.zeros((66, 32), np.float32)
    mlp_rhs[:64] = W4.T
    mlp_rhs[64] = b4 * 0.5
    mlp_rhs[65] = b4 * 0.5
    w5rep = np.tile(W5.reshape(1, 32), (128, 1)).astype(np.float32)
    b5rep = np.full((128, 1), b5[0], np.float32)

    idxk = _prep_idx_flat(indexes)  # [B, 384] int32, pads -1
    # node -> row permutation: node n -> row n-1, node 0 -> row 127.
    # P[b, r, j] = (adj[b, j] == r); pads (-1) give zero columns.
    adj = np.where(idxk < 0, -1, np.where(idxk == 0, 127, idxk - 1))
    onehot = (adj[:, None, :] == np.arange(128, dtype=np.int32)[None, :, None])
    onehot = onehot.astype(bf)  # [B, 128, 384]
    # layer-1 im2col on host: g1[b, c, j] = trees[b, c, idx[j]] (pads -> 0)
    idx0 = np.where(idxk < 0, 0, idxk)  # node 0 is the zero column
    g1pre = np.take_along_axis(
        trees, idx0[:, None, :], axis=2).astype(bf)  # [B, 128, 384]

    in_maps = []
    for c in range(N_CORES):
        lo, hi = c * BC, (c + 1) * BC
        g1c = np.ascontiguousarray(
            g1pre[lo:hi].transpose(1, 0, 2))  # [128, BC, 384]
        ponehot = np.ascontiguousarray(
            onehot[lo:hi].transpose(1, 0, 2))  # [128, BC, 384]
        in_maps.append({
            "g1pre": g1c,
            "ponehot": ponehot,
            "w1r": w1r, "w2t": w2t, "w3t": w3t,
            "biasrows": biasrows, "onescol": onescol, "bcols": bcols,
            "ones128b": ones128b, "ones128f": ones128f,
            "mlp_rhs": mlp_rhs, "w5rep": w5rep, "b5rep": b5rep,
        })

    global _LAST_IN_MAPS
    _LAST_IN_MAPS = in_maps
    res = run_bass_kernel_spmd(nc, in_maps, list(range(N_CORES)))
    out = np.concatenate([res.results[c]["out"] for c in range(N_CORES)], axis=0)
    return out.astype(np.float32)


_LAST_IN_MAPS = None


# revision 44
# speedup vs baseline: 1.1331x; 1.1331x over previous
"""Trainium2 Bass kernel for nn_BaoCypherNet (tree-conv GNN).

Data-parallel over 8 NeuronCores: each core processes 256 trees.

Per tree, per layer l (C_l -> O_l, kernel 3, stride 3 over gathered triples):
  z[o, m] = sum_k sum_c W_k[o, c] * X[c, idx[3m+k]] + b[o]   (m = 1..127)
  x_cat = [0 | z]  -> layernorm over (O, 128) -> leaky_relu (layers 1, 2)

Node-major ("transposed") pipeline -- activations live as [node, channel]
tiles so the on-chip gathers are one-hot matmuls with NO transposes:
  - Layer 1's gather depends only on the input, so it is im2col'd on the
    host: g1[c, j] = trees[c, idx[j]] is DMA'd directly (bf16, k-major
    j = k*128 + m, pad columns zero).
  - Layers 2/3 gather on the PE: g[c, j] = sum_n x[n, c] * P[n, j] with
    stationary = the node-major activation slice, moving = the
    host-built one-hot P[n, j] = (n == idx[j]) (bf16, pad cols zero,
    node n -> row n-1, node 0 -> row 127).  The PSUM result is copied
    to SBUF on the scalar engine (a few per group on DVE for balance).
  - convs (layers 1/2) run "reversed": stationary = gathered k-block
    g_k[c, m], moving = weights W_k[c, o]; output z^T[m, o] lands
    node-major, exactly the layout the next gather wants.  Layer 3 uses
    channel-major convs (stationary weights, N=512 over G=4 trees) since
    its output feeds the free-dim max-pool, not another gather.
  - bias = K=1 matmul (ones column [1,127] stationary) over node rows
    only; LN mean-subtract = one K=128 matmul per bank (ones stationary,
    the (-1/K)-scaled per-row sums broadcast as moving operand).  Since
    the pad columns are zero, PSUM row 127 is exactly -mean after the
    shift, so the full-tile Prelu writes leaky(-mean) -- the node-0
    value -- into activation row 127 for free.
  - LN *scale* is deferred for layers 1/2 (leaky(s*x) = s*leaky(x), next
    LN is scale-invariant up to eps); only layer 3 materializes stats,
    with b3 folded in via activation-bias on the Square and pooled path.
  - Pooling: max over PSUM cols, max(x+b3,0) handles the node-0 column;
    tiny MLP (W4 -> leaky -> W5) finishes on PE+DVE.
"""

import ml_dtypes
import numpy as np

import bass_rust as _bass_rust
import concourse.bass as bass
import concourse.mybir as mybir
from concourse.bass_utils import run_bass_kernel_spmd
from concourse.tile import TileContext

F32 = mybir.dt.float32
BF16 = mybir.dt.bfloat16

N_CORES = 8
B = 2048
BC = B // N_CORES  # trees per core
N = 128            # nodes (incl. zero-pad node 0)
M = N - 1          # conv output positions
G = 4              # trees per group
NGROUPS = BC // G
K1 = 256 * 128     # LN element counts per tree
K2 = 128 * 128
K3 = 64 * 128

_ALPHA = 0.01


def _ap(t_ap, extra_dims, offset_delta=0):
    """Build an AP on the same tensor with given free dims appended to the
    partition dim of `t_ap` (a full-tile AP)."""
    return bass.AP(
        tensor=t_ap.tensor,
        offset=t_ap.offset + offset_delta,
        ap=[t_ap.ap[0]] + list(extra_dims),
    )


def build_nc():
    nc = bass.Bass()

    g1_in = nc.dram_tensor("g1pre", [128, BC, 384], BF16, kind="ExternalInput")
    p_in = nc.dram_tensor("ponehot", [128, BC, 384], BF16, kind="ExternalInput")
    w1_in = nc.dram_tensor("w1r", [128, 3, 256], BF16, kind="ExternalInput")
    w2_in = nc.dram_tensor("w2t", [128, 3, 2, 128], BF16, kind="ExternalInput")
    w3_in = nc.dram_tensor("w3t", [128, 3, 64], BF16, kind="ExternalInput")
    br_in = nc.dram_tensor("biasrows", [1, 384], BF16, kind="ExternalInput")
    oc_in = nc.dram_tensor("onescol", [1, 128], BF16, kind="ExternalInput")
    bc_in = nc.dram_tensor("bcols", [128, 8], F32, kind="ExternalInput")
    ones128b_in = nc.dram_tensor("ones128b", [128, 128], BF16, kind="ExternalInput")
    ones128f_in = nc.dram_tensor("ones128f", [128, 128], F32, kind="ExternalInput")
    mlp_in = nc.dram_tensor("mlp_rhs", [66, 32], F32, kind="ExternalInput")
    w5_in = nc.dram_tensor("w5rep", [128, 32], F32, kind="ExternalInput")
    b5_in = nc.dram_tensor("b5rep", [128, 1], F32, kind="ExternalInput")
    out_dram = nc.dram_tensor("out", [BC, 1], F32, kind="ExternalOutput")

    with TileContext(nc) as tc:
        with (
            tc.tile_pool(name="const", bufs=1) as cp,
            tc.tile_pool(name="sb", bufs=4) as sb,
            tc.tile_pool(name="psz1", bufs=1, space="PSUM") as psz1,
            tc.tile_pool(name="pszB", bufs=1, space="PSUM") as pszB,
            tc.tile_pool(name="psg", bufs=4, space="PSUM") as psg,
            tc.tile_pool(name="pss", bufs=1, space="PSUM") as pss,
        ):
            # ---- constants ----
            w1r = cp.tile([128, 3, 256], BF16, tag="w1r")
            nc.sync.dma_start(out=w1r[:], in_=w1_in[:])
            w2t = cp.tile([128, 3, 2, 128], BF16, tag="w2t")
            nc.sync.dma_start(out=w2t[:], in_=w2_in[:])
            w3t = cp.tile([128, 3, 64], BF16, tag="w3t")
            nc.sync.dma_start(out=w3t[:], in_=w3_in[:])
            biasrows = cp.tile([1, 384], BF16, tag="biasrows")
            nc.sync.dma_start(out=biasrows[:], in_=br_in[:])
            onescol = cp.tile([1, 128], BF16, tag="onescol")
            nc.sync.dma_start(out=onescol[:], in_=oc_in[:])
            bcols = cp.tile([128, 8], F32, tag="bcols")
            nc.sync.dma_start(out=bcols[:], in_=bc_in[:])
            ones128b = cp.tile([128, 128], BF16, tag="ones128b")
            nc.sync.dma_start(out=ones128b[:], in_=ones128b_in[:])
            ones128f = cp.tile([128, 128], F32, tag="ones128f")
            nc.sync.dma_start(out=ones128f[:], in_=ones128f_in[:])
            mlp_rhs = cp.tile([66, 32], F32, tag="mlp_rhs")
            nc.sync.dma_start(out=mlp_rhs[:], in_=mlp_in[:])
            w5rep = cp.tile([128, 32], F32, tag="w5rep")
            nc.sync.dma_start(out=w5rep[:], in_=w5_in[:])
            b5rep = cp.tile([128, 1], F32, tag="b5rep")
            nc.sync.dma_start(out=b5rep[:], in_=b5_in[:])

            for g in range(NGROUPS):
                t0 = g * G
                # ---- inputs for this group ----
                # g1 is host-gathered (im2col): g1[c, t, j] = x1[c, idx[j]]
                g1 = sb.tile([128, G, 384], BF16, tag="g1")
                nc.sync.dma_start(out=g1[:], in_=g1_in[:, t0:t0 + G, :])
                P = sb.tile([128, G, 384], BF16, tag="P")
                nc.sync.dma_start(out=P[:], in_=p_in[:, t0:t0 + G, :])

                # ---- layer 1 convs (reversed): z1T[m, o] per tree ----
                z1T = psz1.tile([128, G, 256], F32, tag="z1")
                for t in range(G):
                    for k in range(3):
                        nc.tensor.matmul(
                            z1T[:, t, :],
                            g1[:, t, k * 128:(k + 1) * 128],
                            w1r[:, k, :],
                            start=(t % 2 == 0 and k == 0), stop=(k == 2),
                            skip_group_check=True)

                # LN1 mean: per-row sums -> fused -mean via ones MM
                s1 = sb.tile([128, G], F32, tag="s1")
                nc.vector.tensor_reduce(
                    s1[:], _ap(z1T[:], [[256, G], [1, 256]]),
                    axis=mybir.AxisListType.X, op=mybir.AluOpType.add)
                s1n = sb.tile([128, G], BF16, tag="s1n")
                with nc.allow_low_precision(reason="mean shift in bf16"):
                    nc.vector.tensor_scalar(
                        s1n[:], s1[:], -1.0 / K1, bcols[:, 5:6],
                        mybir.AluOpType.mult, mybir.AluOpType.add)
                # bias (rows 0..126 only) + mean shift (all 128 rows)
                z1Tr = z1T[0:127, :, :]
                for h in range(2):
                    nc.tensor.matmul(
                        bass.AP(tensor=z1Tr.tensor,
                                offset=z1Tr.offset + h * 512,
                                ap=[z1Tr.ap[0], [256, 2], [1, 256]]),
                        onescol[:, 0:127],
                        _ap(biasrows[:], [[0, 2], [1, 256]]),
                        start=False, stop=False, skip_group_check=True)
                    nc.tensor.matmul(
                        _ap(z1T[:], [[256, 2], [1, 256]], h * 512),
                        ones128b[:],
                        bass.AP(tensor=s1n.tensor,
                                offset=s1n[:].offset + h * 2,
                                ap=[s1n[:].ap[0], [1, 2], [0, 256]]),
                        start=False, stop=True, skip_group_check=True)

                # X2 node-major [row, t, c]: row r = node r+1 for r<127;
                # row 127 = node 0 = leaky(-mean) (PSUM pad row, no bias).
                x2 = sb.tile([128, G, 256], BF16, tag="x2")
                nc.scalar.activation(
                    x2[:], z1T[:],
                    mybir.ActivationFunctionType.Prelu,
                    bias=0.0, scale=1.0, alpha=_ALPHA)

                # ---- layer 2 gathers (stationary = x2 chunks) ----
                g2 = sb.tile([128, 2, 3, G, 128], BF16, tag="g2")
                for t in range(G):
                    for j in range(2):
                        gp = psg.tile([128, 384], F32, tag="g",
                                      name=f"g2p{t}{j}")
                        nc.tensor.matmul(
                            gp[:], x2[:, t, j * 128:(j + 1) * 128],
                            P[:, t, :], start=True, stop=True)
                        g2out = _ap(g2[:], [[G * 128, 3], [1, 128]],
                                    j * 3 * G * 128 + t * 128)
                        if t == 3:
                            with nc.allow_low_precision(reason="copy bf16"):
                                nc.vector.tensor_copy(g2out, gp[:])
                        else:
                            nc.scalar.activation(
                                g2out, gp[:],
                                mybir.ActivationFunctionType.Copy,
                                bias=0.0, scale=1.0)

                # ---- layer 2 convs (reversed): z2T[m, o] ----
                z2T = pszB.tile([128, G, 128], F32, tag="zB", name="z2T")
                for t in range(G):
                    for j in range(2):
                        for k in range(3):
                            nc.tensor.matmul(
                                z2T[:, t, :], g2[:, j, k, t, :],
                                w2t[:, k, j, :],
                                start=(t == 0 and j == 0 and k == 0),
                                stop=(k == 2 and j == 1),
                                skip_group_check=True)

                # LN2 mean
                s2 = sb.tile([128, G], F32, tag="s2")
                nc.vector.tensor_reduce(
                    s2[:], _ap(z2T[:], [[128, G], [1, 128]]),
                    axis=mybir.AxisListType.X, op=mybir.AluOpType.add)
                s2n = sb.tile([128, G], BF16, tag="s2n")
                with nc.allow_low_precision(reason="mean shift in bf16"):
                    nc.vector.tensor_scalar(
                        s2n[:], s2[:], -1.0 / K2, bcols[:, 6:7],
                        mybir.AluOpType.mult, mybir.AluOpType.add)
                z2Tr = z2T[0:127, :, :]
                nc.tensor.matmul(
                    bass.AP(tensor=z2Tr.tensor, offset=z2Tr.offset,
                            ap=[z2Tr.ap[0], [128, G], [1, 128]]),
                    onescol[:, 0:127],
                    _ap(biasrows[:], [[0, G], [1, 128]], 256),
                    start=False, stop=False, skip_group_check=True)
                nc.tensor.matmul(
                    _ap(z2T[:], [[128, G], [1, 128]]),
                    ones128b[:],
                    _ap(s2n[:], [[1, G], [0, 128]]),
                    start=False, stop=True, skip_group_check=True)

                # X3 node-major, same row permutation as X2
                x3 = sb.tile([128, G, 128], BF16, tag="x3")
                nc.scalar.activation(
                    x3[:], z2T[:],
                    mybir.ActivationFunctionType.Prelu,
                    bias=0.0, scale=1.0, alpha=_ALPHA)

                # ---- layer 3: gather, channel-major conv, LN, max-pool ----
                g3 = sb.tile([128, 3, G, 128], BF16, tag="g3")
                for t in range(G):
                    gp = psg.tile([128, 384], F32, tag="g", name=f"g3p{t}")
                    nc.tensor.matmul(gp[:], x3[:, t, :], P[:, t, :],
                                     start=True, stop=True)
                    g3out = _ap(g3[:], [[G * 128, 3], [1, 128]], t * 128)
                    if t >= 2:
                        with nc.allow_low_precision(reason="copy bf16"):
                            nc.vector.tensor_copy(g3out, gp[:])
                    else:
                        nc.scalar.activation(
                            g3out, gp[:], mybir.ActivationFunctionType.Copy,
                            bias=0.0, scale=1.0)

                z3 = pszB.tile([64, G, 128], F32, tag="zB", name="z3")
                z3_all = _ap(z3[:], [[128, G], [1, 128]])
                for k in range(3):
                    rhs = _ap(g3[:], [[128, G], [1, 128]], k * G * 128)
                    nc.tensor.matmul(z3_all, w3t[:, k, :], rhs,
                                     start=(k == 0), stop=(k == 2))

                # LN3 stats on z3+b3 (b3 via ACT bias / post-corrections)
                z3v = _ap(z3[:], [[128, G], [1, 127]])
                s3 = sb.tile([64, 2, G], F32, tag="s3")
                nc.vector.tensor_reduce(
                    s3[:, 0, :], z3v, axis=mybir.AxisListType.X,
                    op=mybir.AluOpType.add)
                nc.vector.tensor_scalar(
                    s3[:, 0, :], s3[:, 0, :], bcols[0:64, 4:5], None,
                    mybir.AluOpType.add)
                sq = sb.tile([64, G, 127], F32, tag="sq")
                nc.scalar.activation(
                    sq[:], z3v, mybir.ActivationFunctionType.Square,
                    bias=bcols[0:64, 3:4], scale=1.0)
                nc.vector.tensor_reduce(
                    s3[:, 1, :], sq[:], axis=mybir.AxisListType.X,
                    op=mybir.AluOpType.add)
                ps3 = pss.tile([128, 2, G], F32, tag="pss", name="ps3")
                nc.tensor.matmul(
                    ps3[:], ones128f[0:64, :],
                    bass.AP(tensor=s3.tensor, offset=s3[:].offset,
                            ap=[s3[:].ap[0], [1, 2 * G]]),
                    start=True, stop=True)
                # mean3 = S/K3; nm3 = -mean3; var = SS/(K3-1) - K3/(K3-1)*mean^2
                mean3 = sb.tile([128, G], F32, tag="mean3")
                nc.vector.tensor_scalar(
                    mean3[:], ps3[:, 0, :], 1.0 / K3, None, mybir.AluOpType.mult)
                nm3 = sb.tile([128, G], F32, tag="nm3")
                nc.vector.tensor_scalar(
                    nm3[:], mean3[:], -1.0, None, mybir.AluOpType.mult)
                m3sq = sb.tile([128, G], F32, tag="m3sq")
                nc.vector.tensor_tensor(
                    m3sq[:], mean3[:], mean3[:], mybir.AluOpType.mult)
                var3 = sb.tile([128, G], F32, tag="var3")
                nc.vector.tensor_scalar(
                    var3[:], m3sq[:], -float(K3) / (K3 - 1), None,
                    mybir.AluOpType.mult)
                ssn = sb.tile([128, G], F32, tag="ssn")
                nc.vector.tensor_scalar(
                    ssn[:], ps3[:, 1, :], 1.0 / (K3 - 1), None,
                    mybir.AluOpType.mult)
                nc.vector.tensor_tensor(
                    var3[:], var3[:], ssn[:], mybir.AluOpType.add)
                std3 = sb.tile([128, G], F32, tag="std3")
                nc.scalar.activation(
                    std3[:], var3[:], mybir.ActivationFunctionType.Sqrt,
                    bias=0.0, scale=1.0)
                nc.vector.tensor_scalar(
                    std3[:], std3[:], 1e-5, None, mybir.AluOpType.add)
                sinv3 = sb.tile([128, G], F32, tag="sinv3")
                nc.vector.reciprocal(sinv3[:], std3[:])

                # pooled = sinv3 * (max(max_m z3 + b3, 0) - mean3)
                pr = sb.tile([64, G], F32, tag="pr")
                nc.vector.tensor_reduce(
                    pr[:], z3v, axis=mybir.AxisListType.X,
                    op=mybir.AluOpType.max)
                paug = sb.tile([66, G], F32, tag="paug")
                nc.vector.memset(paug[64:66, :], 1.0)
                r1 = sb.tile([64, G], F32, tag="r1")
                nc.vector.tensor_scalar(
                    r1[:], pr[:], bcols[0:64, 3:4], 0.0,
                    mybir.AluOpType.add, mybir.AluOpType.max)
                r2 = sb.tile([64, G], F32, tag="r2")
                nc.vector.tensor_tensor(
                    r2[:], r1[:], nm3[0:64, :], mybir.AluOpType.add)
                nc.vector.tensor_tensor(
                    paug[0:64, :], r2[:], sinv3[0:64, :], mybir.AluOpType.mult)

                # h = leaky(W4 @ pooled + b4); out = h @ W5.T + b5
                ph = pss.tile([G, 32], F32, tag="pss", name="ph")
                nc.tensor.matmul(ph[:], paug[:, :], mlp_rhs[0:66, :],
                                 start=True, stop=True)
                h = sb.tile([G, 32], F32, tag="h")
                nc.scalar.activation(
                    h[:], ph[:], mybir.ActivationFunctionType.Prelu,
                    bias=0.0, scale=1.0, alpha=_ALPHA)
                prod = sb.tile([G, 32], F32, tag="prod")
                nc.vector.tensor_tensor(
                    prod[:], h[:], w5rep[0:G, :], mybir.AluOpType.mult)
                ov = sb.tile([G, 1], F32, tag="ov")
                nc.vector.tensor_reduce(
                    ov[:], prod[:], axis=mybir.AxisListType.X,
                    op=mybir.AluOpType.add)
                nc.vector.tensor_scalar(
                    ov[:], ov[:], b5rep[0:G, :], None, mybir.AluOpType.add)
                nc.sync.dma_start(out=out_dram[t0:t0 + G, :], in_=ov[:])

    _bass_rust.generate_event_semaphores(nc)
    nc.finalize()
    return nc


_NC_CACHE = None


def _get_nc():
    global _NC_CACHE
    if _NC_CACHE is None:
        _NC_CACHE = build_nc()
    return _NC_CACHE


def _prep_idx_flat(indexes: np.ndarray) -> np.ndarray:
    """indexes [B, 381] -> k-major [B, 384] int32 with pads = -1."""
    b = indexes.shape[0]
    idxk = np.full((b, 3, 128), -1, np.int32)
    tri = indexes.reshape(b, 127, 3).astype(np.int32)
    idxk[:, :, :127] = tri.transpose(0, 2, 1)
    return idxk.reshape(b, 384)


def kernel(trees, W1, b1, W2, b2, W3, b3, W4, b4, W5, b5, indexes):
    trees = np.asarray(trees, dtype=np.float32)
    indexes = np.asarray(indexes).astype(np.int64)
    W1 = np.asarray(W1, dtype=np.float32)
    W2 = np.asarray(W2, dtype=np.float32)
    W3 = np.asarray(W3, dtype=np.float32)
    W4 = np.asarray(W4, dtype=np.float32)
    W5 = np.asarray(W5, dtype=np.float32)
    b1 = np.asarray(b1, dtype=np.float32)
    b2 = np.asarray(b2, dtype=np.float32)
    b3 = np.asarray(b3, dtype=np.float32)
    b4 = np.asarray(b4, dtype=np.float32)
    b5 = np.asarray(b5, dtype=np.float32)

    nc = _get_nc()

    bf = ml_dtypes.bfloat16
    # replicated weight prep
    # w1r[c, k, o] = W1[o, c, k]
    w1r = np.ascontiguousarray(W1.transpose(1, 2, 0)).astype(bf)
    # w2t[p, k, j, o] = W2[o, j*128+p, k]
    w2t = np.ascontiguousarray(
        W2.reshape(128, 2, 128, 3).transpose(2, 3, 1, 0)).astype(bf)
    # w3t[c, k, o] = W3[o, c, k]
    w3t = np.ascontiguousarray(W3.transpose(1, 2, 0)).astype(bf)
    biasrows = np.zeros((1, 384), np.float32)
    biasrows[0, :256] = b1
    biasrows[0, 256:] = b2
    biasrows = biasrows.astype(bf)
    onescol = np.ones((1, 128), bf)
    # bias columns + scalar mean-bias corrections
    bcols = np.zeros((128, 8), np.float32)
    bcols[:64, 3] = b3
    bcols[:64, 4] = 127.0 * b3
    bcols[:, 5] = -127.0 * float(b1.sum()) / (K1 * 128)  # L1 shift mean-bias
    bcols[:, 6] = -127.0 * float(b2.sum()) / (K2 * 128)  # L2 shift mean-bias
    ones128b = np.ones((128, 128), bf)
    ones128f = np.ones((128, 128), np.float32)
    mlp_rhs = np.zeros((66, 32), np.float32)
    mlp_rhs[:64] = W4.T
    mlp_rhs[64] = b4 * 0.5
    mlp_rhs[65] = b4 * 0.5
    w5rep = np.tile(W5.reshape(1, 32), (128, 1)).astype(np.float32)
    b5rep = np.full((128, 1), b5[0], np.float32)

    idxk = _prep_idx_flat(indexes)  # [B, 384] int32, pads -1
    # node -> row permutation: node n -> row n-1, node 0 -> row 127.
    # P[b, r, j] = (adj[b, j] == r); pads (-1) give zero columns.
    adj = np.where(idxk < 0, -1, np.where(idxk == 0, 127, idxk - 1))
    onehot = (adj[:, None, :] == np.arange(128, dtype=np.int32)[None, :, None])
    onehot = onehot.astype(bf)  # [B, 128, 384]
    # layer-1 im2col on host: g1[b, c, j] = trees[b, c, idx[j]] (pads -> 0)
    idx0 = np.where(idxk < 0, 0, idxk)  # node 0 is the zero column
    g1pre = np.take_along_axis(
        trees, idx0[:, None, :], axis=2).astype(bf)  # [B, 128, 384]

    in_maps = []
    for c in range(N_CORES):
        lo, hi = c * BC, (c + 1) * BC
        g1c = np.ascontiguousarray(
            g1pre[lo:hi].transpose(1, 0, 2))  # [128, BC, 384]
        ponehot = np.ascontiguousarray(
            onehot[lo:hi].transpose(1, 0, 2))  # [128, BC, 384]
        in_maps.append({
            "g1pre": g1c,
            "ponehot": ponehot,
            "w1r": w1r, "w2t": w2t, "w3t": w3t,
            "biasrows": biasrows, "onescol": onescol, "bcols": bcols,
            "ones128b": ones128b, "ones128f": ones128f,
            "mlp_rhs": mlp_rhs, "w5rep": w5rep, "b5rep": b5rep,
        })

    global _LAST_IN_MAPS
    _LAST_IN_MAPS = in_maps
    res = run_bass_kernel_spmd(nc, in_maps, list(range(N_CORES)))
    out = np.concatenate([res.results[c]["out"] for c in range(N_CORES)], axis=0)
    return out.astype(np.float32)


_LAST_IN_MAPS = None
